# revision 21
# baseline (speedup 1.0000x reference)
"""Tensor-parallel GQA attention block for 8 TRN2 NeuronCores.

Sharding: TP over heads. Core c owns query heads 4c..4c+3 and KV head c
(column-shard of wq/wk/wv), plus the matching column-shard of wo. x is
replicated. Each core computes its partial x@wo_shard.T contribution and a
ReduceScatter sums them, leaving each core with a row-shard of the output;
the host concatenates the shards.

All device compute in bf16 with f32 PSUM accumulation. Host-side input prep:
weight shards are transposed to contraction-major layout, query-head dims are
permuted to (evens, odds) so RoPE halves sit in partition halves.

Scores are computed transposed (S^T = K @ Q^T, [k, q] layout) so the exp'd
probabilities feed the P@V matmul directly as the moving operand; the
1/sqrt(HD) scale is applied inside the exp activation. Softmax denominators
come from an all-ones matmul (sum + broadcast across partitions in one op)
and are applied as a reciprocal multiply. exp() runs without max-subtraction:
scores here are bounded (|score| < ~15), safe in f32.

The wo partials ship to the ReduceScatter in seven chunks (three 512-row for
q-chunks 0-2, then four 128-row pieces for the final q-chunk so the collective
on the critical tail is small), each issued eagerly; the output DMA rides the
GpSimd queue to keep the Sync DMA queues free for compute traffic. wo weight
shards prefetch during phase 1, and the softmax accumulators are
double-buffered so consecutive heads overlap.
"""
import os

import numpy as np
import ml_dtypes

import concourse.mybir as mybir
import concourse.tile as tile
from concourse import bacc
import concourse.bass_utils as _bu
from concourse.bass_utils import run_bass_kernel_spmd
from concourse.masks import make_identity

if os.environ.get("BASS_LDW_OPT") and not getattr(_bu, "_ldw_patched", False):
    _orig_run_command = _bu.run_command

    def _run_command_ldw(argv, **kw):
        argv = ["--enable-ldw-opt=true" if a == "--enable-ldw-opt=false"
                else a for a in argv]
        return _orig_run_command(argv, **kw)

    _bu.run_command = _run_command_ldw
    _bu._ldw_patched = True

N_CORES = 8
B, S, D = 1, 2048, 4096
H, KVH, HD = 32, 8, 128
HL = H // N_CORES          # 4 local q heads
SCALE = HD ** -0.5
P = 128
QC = 512                   # attention q-chunk width
NQC = S // QC              # 4
NKB = S // P               # 16 k-tiles
NDC = D // P               # 32 D-chunks
PW = 512                   # phase-1 s-panel width
NPAN = S // PW             # 4
SHARD = QC // N_CORES      # 64 rows per core per RS chunk

FDT = mybir.dt.float32
BDT = mybir.dt.bfloat16
HDT = mybir.dt.float16
NEG = -1.0e9

LAST_RESULT = None


def _build(mode):
    nc = bacc.Bacc("TRN2", target_bir_lowering=False, debug=False,
                   num_devices=N_CORES)
    xt_ext = nc.dram_tensor("xt", [D, S], BDT, kind="ExternalInput")
    wqt_ext = nc.dram_tensor("wqt", [D, HL * P], BDT, kind="ExternalInput")
    wkt_ext = nc.dram_tensor("wkt", [D, P], BDT, kind="ExternalInput")
    wvt_ext = nc.dram_tensor("wvt", [D, P], BDT, kind="ExternalInput")
    wot_ext = nc.dram_tensor("wot", [HL * P, D], BDT, kind="ExternalInput")
    c2_ext = nc.dram_tensor("c2", [P, S], BDT, kind="ExternalInput")
    s2_ext = nc.dram_tensor("s2", [P, S], BDT, kind="ExternalInput")
    if mode == "causal":
        tri_ext = nc.dram_tensor("tri", [P, 896], FDT, kind="ExternalInput")
    if mode == "mask":
        maskt_ext = nc.dram_tensor("maskt", [S, S], FDT, kind="ExternalInput")
    out_ext = nc.dram_tensor("out", [NQC * SHARD, D], HDT,
                             kind="ExternalOutput")

    Alu = mybir.AluOpType
    Act = mybir.ActivationFunctionType

    with tile.TileContext(nc) as tc:
        with tc.tile_pool(name="persist", bufs=1) as pers:
            # per-panel tiles keep dependency tracking fine-grained so the
            # first attention chunks unblock as soon as their panel's rope
            # is done rather than waiting on the whole projection phase
            qt_ps = [pers.tile([P, HL * PW], BDT, tag=f"qt{p}",
                               name=f"qt{p}") for p in range(NPAN)]
            ones_sb = pers.tile([P, P], BDT, tag="ones")
            nc.vector.memset(ones_sb[:], 1.0)
            ident = pers.tile([P, P], BDT, tag="ident")
            make_identity(nc, ident[:])
            kt_ps = [pers.tile([P, PW], BDT, tag=f"kt{p}",
                               name=f"kt{p}") for p in range(NPAN)]
            v_ps = [pers.tile([P, 4 * P], BDT, tag=f"v{p}",
                              name=f"v{p}") for p in range(NPAN)]
            wot_sbs = [pers.tile([P, D], BDT, tag=f"wot{hc}",
                                 name=f"wot{hc}") for hc in range(HL)]
            if mode == "causal":
                tri_sb = pers.tile([P, 896], FDT, tag="tri")
                nc.sync.dma_start(tri_sb[:], tri_ext[:])

            # ---------------- phase 1: QKV projections + RoPE -------------
            with (
                tc.tile_pool(name="ph1", bufs=1) as ph1,
                tc.tile_pool(name="xstage", bufs=2) as xst,
                tc.tile_pool(name="rsc", bufs=2) as rsc,
                tc.tile_pool(name="ppsum", bufs=1, space="PSUM") as ppsum,
                tc.tile_pool(name="tpsum", bufs=2, space="PSUM") as tpsum,
            ):
                HW = HL * P
                wqt_sbs = [ph1.tile([P, 4 * HW], BDT, tag=f"wqt{g}",
                                    name=f"wqt{g}") for g in range(8)]
                wkt_sbs = [ph1.tile([P, 8 * P], BDT, tag=f"wkt{g}",
                                    name=f"wkt{g}") for g in range(4)]
                wvt_sbs = [ph1.tile([P, 8 * P], BDT, tag=f"wvt{g}",
                                    name=f"wvt{g}") for g in range(4)]
                c2_sb = ph1.tile([P, S], BDT, tag="c2")
                s2_sb = ph1.tile([P, S], BDT, tag="s2")

                def dma_weights(dc):
                    if dc == 0:
                        # dc0 slices land first so the opening matmuls
                        # aren't gated on the full 2MB group transfers
                        HW4 = HL * P
                        nc.sync.dma_start(wqt_sbs[0][:, 0:HW4],
                                          wqt_ext[0:P, :])
                        nc.sync.dma_start(wkt_sbs[0][:, 0:P],
                                          wkt_ext[0:P, :])
                        nc.sync.dma_start(wvt_sbs[0][:, 0:P],
                                          wvt_ext[0:P, :])
                        nc.sync.dma_start(
                            wqt_sbs[0][:, HW4:4 * HW4].rearrange(
                                "p (dc h) -> p dc h", dc=3),
                            wqt_ext[P:4 * P, :].rearrange(
                                "(dc p) h -> p dc h", p=P),
                        )
                        nc.sync.dma_start(
                            wkt_sbs[0][:, P:8 * P].rearrange(
                                "p (dc h) -> p dc h", dc=7),
                            wkt_ext[P:8 * P, :].rearrange(
                                "(dc p) h -> p dc h", p=P),
                        )
                        nc.sync.dma_start(
                            wvt_sbs[0][:, P:8 * P].rearrange(
                                "p (dc h) -> p dc h", dc=7),
                            wvt_ext[P:8 * P, :].rearrange(
                                "(dc p) h -> p dc h", p=P),
                        )
                        return
                    if dc % 4 == 0:
                        g = dc // 4
                        nc.sync.dma_start(
                            wqt_sbs[g][:].rearrange(
                                "p (dc h) -> p dc h", dc=4),
                            wqt_ext[g * 4 * P:(g + 1) * 4 * P, :].rearrange(
                                "(dc p) h -> p dc h", p=P),
                        )
                    if dc % 8 == 0:
                        g = dc // 8
                        nc.sync.dma_start(
                            wkt_sbs[g][:].rearrange(
                                "p (dc h) -> p dc h", dc=8),
                            wkt_ext[g * 8 * P:(g + 1) * 8 * P, :].rearrange(
                                "(dc p) h -> p dc h", p=P),
                        )
                        nc.sync.dma_start(
                            wvt_sbs[g][:].rearrange(
                                "p (dc h) -> p dc h", dc=8),
                            wvt_ext[g * 8 * P:(g + 1) * 8 * P, :].rearrange(
                                "(dc p) h -> p dc h", p=P),
                        )

                swap_mask = list(range(16, 32)) + list(range(16))

                def rope_free_accs(pan, qacc, kacc, vacc):
                    """Phase a: everything that READS the PSUM accumulators,
                    so they free up for the next panel ASAP."""
                    cols = slice(pan * PW, (pan + 1) * PW)
                    state = []
                    for i, acc in enumerate(qacc + [kacc]):
                        t_sb = rsc.tile([P, PW], BDT, tag=f"ropea{i}",
                                        name="t_sb")
                        nc.scalar.copy(t_sb[:], acc[:])
                        tsw = rsc.tile([P, PW], BDT, tag=f"ropet{i}",
                                       name="tsw")
                        nc.sync.dma_start(tsw[0:64, :], t_sb[64:128, :])
                        nc.sync.dma_start(tsw[64:128, :], t_sb[0:64, :])
                        m = rsc.tile([P, PW], FDT, tag=f"ropem{i}", name="m")
                        nc.vector.tensor_tensor(m[:], acc[:],
                                                c2_sb[:, cols], op=Alu.mult)
                        state.append((tsw, m))
                    vtmp = rsc.tile([P, PW], BDT, tag="vtmp")
                    nc.scalar.copy(vtmp[:], vacc[:])
                    return (pan, state, vtmp)

                def rope_panel(pan, state, vtmp):
                    cols = slice(pan * PW, (pan + 1) * PW)
                    outs = [qt_ps[pan][:, h * PW:(h + 1) * PW]
                            for h in range(HL)]
                    outs.append(kt_ps[pan][:])
                    for (tsw, m), out in zip(state, outs):
                        n = rsc.tile([P, PW], FDT, tag="ropen")
                        nc.vector.tensor_tensor(n[:], tsw[:],
                                                s2_sb[:, cols], op=Alu.mult)
                        nc.vector.tensor_tensor(out, m[:], n[:], op=Alu.add)
                    vtp = tpsum.tile([P, PW], BDT, tag="xtp", name="vtp")
                    for st4 in range(4):
                        nc.tensor.transpose(
                            vtp[:, st4 * P:(st4 + 1) * P],
                            vtmp[:, st4 * P:(st4 + 1) * P], ident[:])
                    nc.scalar.copy(v_ps[pan][:], vtp[:])

                prev = None
                for pan in range(NPAN):
                    if prev is not None:
                        prev = rope_free_accs(*prev)
                    xts_tiles = []
                    for dc in range(NDC):
                        xts = xst.tile([P, PW], BDT, tag="xts", bufs=33)
                        nc.sync.dma_start(
                            xts[:],
                            xt_ext[dc * P:(dc + 1) * P,
                                   pan * PW:(pan + 1) * PW])
                        if pan == 0:
                            dma_weights(dc)
                        xts_tiles.append(xts)
                    if pan == 0:
                        nc.sync.dma_start(c2_sb[:], c2_ext[:])
                        nc.sync.dma_start(s2_sb[:], s2_ext[:])
                    if pan == 1:
                        # prefetch wo shards while the DMA queues have slack
                        for hc in range(HL):
                            nc.sync.dma_start(
                                wot_sbs[hc][:],
                                wot_ext[hc * P:(hc + 1) * P, :])
                    if prev is not None:
                        rope_panel(*prev)
                    qacc = [ppsum.tile([P, PW], FDT, tag=f"qacc{h}",
                                       name=f"qacc{h}")
                            for h in range(HL)]
                    kacc = ppsum.tile([P, PW], FDT, tag="kacc")
                    vacc = ppsum.tile([P, PW], FDT, tag="vacc")
                    for dc in range(NDC):
                        xts = xts_tiles[dc]
                        first, last = dc == 0, dc == NDC - 1
                        wq_t = wqt_sbs[dc // 4]
                        dq = dc % 4
                        for h in range(HL):
                            nc.tensor.matmul(
                                qacc[h][:],
                                wq_t[:, dq * HW + h * P:
                                     dq * HW + (h + 1) * P],
                                xts[:], start=first, stop=last)
                        nc.tensor.matmul(
                            kacc[:],
                            wkt_sbs[dc // 8][:, (dc % 8) * P:
                                             (dc % 8 + 1) * P],
                            xts[:], start=first, stop=last)
                        nc.tensor.matmul(
                            vacc[:],
                            wvt_sbs[dc // 8][:, (dc % 8) * P:
                                             (dc % 8 + 1) * P],
                            xts[:], start=first, stop=last)
                    prev = (pan, qacc, kacc, vacc)
                rope_panel(*rope_free_accs(*prev))

            # ---------------- phase 2: attention + wo + ReduceScatter -----
            with (
                tc.tile_pool(name="att", bufs=2) as att,
                tc.tile_pool(name="ptpool", bufs=4) as ptp,
                tc.tile_pool(name="scr", bufs=2) as scp,
                tc.tile_pool(name="mtpool", bufs=17) as mtp,
                tc.tile_pool(name="cdram", bufs=4, space="DRAM") as cdram,
                tc.tile_pool(name="stps", bufs=2, space="PSUM") as stps,
                tc.tile_pool(name="avps", bufs=2, space="PSUM") as avps,
                tc.tile_pool(name="wops", bufs=2, space="PSUM") as wops,
            ):
                def attn_chunk(qabs0, qw, mts):
                    """Attention for q columns [qabs0, qabs0+qw); returns
                    per-head normalized (P@V)^T tiles (valid cols [0, qw))."""
                    nkb_ = ((qabs0 + qw) // P if mode == "causal" else NKB)
                    qpan, qpoff = divmod(qabs0, PW)
                    attn_t = []
                    for h in range(HL):
                        ssum = avps.tile([P, QC], FDT, tag="ssum", bufs=2)
                        avt = avps.tile([P, QC], FDT, tag="avt", bufs=2)
                        q0 = h * PW + qpoff
                        for kb in range(nkb_):
                            # causal: piece columns below c0 are fully masked
                            c0 = (max(0, kb * P - qabs0)
                                  if mode == "causal" else 0)
                            st_ps = stps.tile([P, QC], FDT, tag="st")
                            nc.tensor.matmul(
                                st_ps[:, c0:qw],
                                kt_ps[kb // 4][:, (kb % 4) * P:
                                               (kb % 4 + 1) * P],
                                qt_ps[qpan][:, q0 + c0:q0 + qw],
                                start=True, stop=True)
                            if mode == "causal" and kb * P >= qabs0:
                                nc.vector.tensor_tensor(
                                    st_ps[:, c0:c0 + P], st_ps[:, c0:c0 + P],
                                    tri_sb[:, 384:384 + P],
                                    op=Alu.add)
                            elif mode == "mask":
                                nc.vector.tensor_tensor(
                                    st_ps[:, :qw], st_ps[:, :qw],
                                    mts[kb][:, :qw], op=Alu.add)
                            pt = ptp.tile([P, QC], BDT, tag="pt")
                            nc.scalar.activation(pt[:, c0:qw],
                                                 st_ps[:, c0:qw], Act.Exp,
                                                 scale=float(SCALE))
                            first, last = kb == 0, kb == nkb_ - 1
                            nc.tensor.matmul(
                                ssum[:, c0:qw], ones_sb[:], pt[:, c0:qw],
                                start=first, stop=last)
                            nc.tensor.matmul(
                                avt[:, c0:qw],
                                v_ps[kb // 4][:, (kb % 4) * P:
                                              (kb % 4 + 1) * P],
                                pt[:, c0:qw],
                                start=first, stop=last)
                        rsb = scp.tile([P, QC], FDT, tag="rsb")
                        nc.vector.reciprocal_approx_fast(out=rsb[:, :qw],
                                                         in_=ssum[:, :qw])
                        at = att.tile([P, QC], BDT, tag=f"attnT{h}",
                                      name=f"attnT{h}")
                        nc.vector.tensor_tensor(at[:, :qw], avt[:, :qw],
                                                rsb[:, :qw], op=Alu.mult)
                        attn_t.append(at)
                    return attn_t

                def wo_rs(attn_t, col0, cnt, out_off, last=False):
                    """wo partials for cnt 128-row blocks of attn_t starting
                    at column col0, then an eagerly-issued ReduceScatter
                    straight to the output."""
                    hrows = cnt * P
                    rs_in = cdram.tile([hrows, D], HDT, tag="rsin",
                                       bufs=4, name="rs_in",
                                       padded_shape=[QC, D])
                    for st2 in range(cnt):
                        rs_full = scp.tile([P, D], HDT, tag="rsfull",
                                           bufs=4, name="rs_full")
                        for do in range(8):
                            ops = wops.tile([P, QC], FDT, tag="wops",
                                            bufs=2)
                            for hc in range(HL):
                                nc.tensor.matmul(
                                    ops[:],
                                    attn_t[hc][:, col0 + st2 * P:
                                               col0 + (st2 + 1) * P],
                                    wot_sbs[hc][:, do * QC:(do + 1) * QC],
                                    start=(hc == 0), stop=(hc == HL - 1))
                            nc.vector.tensor_copy(
                                out=rs_full[:, do * QC:(do + 1) * QC],
                                in_=ops[:])
                        # one 8KB-per-partition DMA per 128 rows: big
                        # contiguous descriptors survive RS ring contention
                        nc.sync.dma_start(
                            rs_in[st2 * P:(st2 + 1) * P, :], rs_full[:])
                    nr = hrows // N_CORES
                    rs_out = cdram.tile([nr, D], HDT, tag="rsout", bufs=4,
                                        name="rs_out",
                                        padded_shape=[QC // N_CORES, D])
                    nc.gpsimd.collective_compute(
                        "ReduceScatter", Alu.add,
                        ins=[rs_in[:]], outs=[rs_out[:]],
                        replica_groups=[list(range(N_CORES))])
                    # the final piece's output DMA rides the HWDGE sync
                    # queue (lower first-byte latency on the critical tail)
                    eng = nc.sync if last else nc.gpsimd
                    eng.dma_start(
                        out_ext[out_off:out_off + nr, :], rs_out[:])
                    return out_off + nr

                out_off = 0
                for qc in range(NQC):
                    # the final q-chunk runs in 256-row pieces so the last
                    # ReduceScatter on the critical tail is small (256 keeps
                    # the matmul free dim above the LDWEIGHTS-bound regime)
                    pieces = ([(0, QC)] if qc != NQC - 1
                              else [(0, 2 * P), (2 * P, 2 * P)])
                    for qoff, qw in pieces:
                        qabs0 = qc * QC + qoff
                        mts = []
                        if mode == "mask":
                            for kb in range(NKB):
                                mt = mtp.tile([P, QC], FDT, tag="mt",
                                              name="mt")
                                nc.sync.dma_start(
                                    mt[:, :qw],
                                    maskt_ext[kb * P:(kb + 1) * P,
                                              qabs0:qabs0 + qw])
                                mts.append(mt)
                        attn_t = attn_chunk(qabs0, qw, mts)
                        if qc == NQC - 1 and qoff == 2 * P:
                            # last attention piece: two 128-row RS chunks so
                            # the tail collective is as small as possible
                            out_off = wo_rs(attn_t, 0, 1, out_off)
                            out_off = wo_rs(attn_t, P, 1, out_off, last=True)
                        else:
                            out_off = wo_rs(attn_t, 0, qw // P, out_off)
    nc.compile()
    return nc


def _prep_inputs(x, freqs_cos, freqs_sin, mask, wq, wk, wv, wo, mode):
    bf16 = ml_dtypes.bfloat16
    perm = np.concatenate([np.arange(0, HD, 2), np.arange(1, HD, 2)])
    xt = np.ascontiguousarray(x.reshape(S, D).T.astype(bf16))
    cosT = np.ascontiguousarray(freqs_cos.T, dtype=np.float32)  # (64, S)
    sinT = np.ascontiguousarray(freqs_sin.T, dtype=np.float32)
    c2 = np.ascontiguousarray(np.vstack([cosT, cosT]).astype(bf16))
    s2 = np.ascontiguousarray(np.vstack([-sinT, sinT]).astype(bf16))
    t = np.arange(896) - 384
    tri = np.where(t[None, :] >= np.arange(P)[:, None], 0.0,
                   NEG / SCALE).astype(np.float32)
    wq4 = wq.reshape(H, HD, D)[:, perm, :]
    wk4 = wk.reshape(KVH, HD, D)[:, perm, :]
    wv4 = wv.reshape(KVH, HD, D)
    in_maps = []
    for c in range(N_CORES):
        wqs = wq4[c * HL:(c + 1) * HL].reshape(HL * HD, D)
        m = {
            "xt": xt,
            "wqt": np.ascontiguousarray(wqs.T).astype(bf16),
            "wkt": np.ascontiguousarray(wk4[c].T).astype(bf16),
            "wvt": np.ascontiguousarray(wv4[c].T).astype(bf16),
            "wot": np.ascontiguousarray(
                wo[:, c * HL * HD:(c + 1) * HL * HD].T).astype(bf16),
            "c2": c2, "s2": s2,
        }
        if mode == "causal":
            m["tri"] = tri
        if mode == "mask":
            m["maskt"] = np.ascontiguousarray(
                mask.T / SCALE, dtype=np.float32)
        in_maps.append(m)
    return in_maps


def _mask_mode(mask):
    if np.all(mask == 0):
        return "zeros"
    iu = np.triu_indices(S, 1)
    if (np.all(np.tril(mask) == 0) and np.all(mask[iu] <= -1e8)
            and np.all(mask[iu] >= -2e9)):
        return "causal"
    return "mask"


_GRAPH_CACHE = {}


def kernel(x, freqs_cos, freqs_sin, mask, wq, wk, wv, wo):
    global LAST_RESULT
    mode = _mask_mode(np.asarray(mask))
    if mode not in _GRAPH_CACHE:
        _GRAPH_CACHE[mode] = _build(mode)
    nc = _GRAPH_CACHE[mode]
    in_maps = _prep_inputs(
        np.asarray(x), np.asarray(freqs_cos), np.asarray(freqs_sin),
        np.asarray(mask), np.asarray(wq), np.asarray(wk), np.asarray(wv),
        np.asarray(wo), mode)
    res = run_bass_kernel_spmd(
        nc, in_maps, core_ids=list(range(N_CORES)),
        trace=bool(os.environ.get("BASS_TRACE")))
    LAST_RESULT = res
    out = np.empty((S, D), dtype=np.float32)
    chunks = [(0, 512), (512, 512), (1024, 512),
              (1536, 256), (1792, 128), (1920, 128)]
    for c in range(N_CORES):
        shard = np.asarray(res.results[c]["out"], dtype=np.float32)
        off = 0
        for src_row0, nrows in chunks:
            nr = nrows // N_CORES
            out[src_row0 + c * nr: src_row0 + (c + 1) * nr] = \
                shard[off:off + nr]
            off += nr
    return out.reshape(B, S, D)



# revision 23
# speedup vs baseline: 1.0273x; 1.0273x over previous
"""Tensor-parallel GQA attention block for 8 TRN2 NeuronCores.

Sharding: TP over heads. Core c owns query heads 4c..4c+3 and KV head c
(column-shard of wq/wk/wv), plus the matching column-shard of wo. x is
replicated. Each core computes its partial x@wo_shard.T contribution and a
ReduceScatter sums them, leaving each core with a row-shard of the output;
the host concatenates the shards.

All device compute in bf16 with f32 PSUM accumulation. Host-side input prep:
weight shards are transposed to contraction-major layout, query-head dims are
permuted to (evens, odds) so RoPE halves sit in partition halves.

Scores are computed transposed (S^T = K @ Q^T, [k, q] layout) so the exp'd
probabilities feed the P@V matmul directly as the moving operand; the
1/sqrt(HD) scale is applied inside the exp activation. Softmax denominators
come from an all-ones matmul (sum + broadcast across partitions in one op)
and are applied as a reciprocal multiply. exp() runs without max-subtraction:
scores here are bounded (|score| < ~15), safe in f32.

The wo partials ship to the ReduceScatter in seven chunks (three 512-row for
q-chunks 0-2, then four 128-row pieces for the final q-chunk so the collective
on the critical tail is small), each issued eagerly; the output DMA rides the
GpSimd queue to keep the Sync DMA queues free for compute traffic. wo weight
shards prefetch during phase 1, and the softmax accumulators are
double-buffered so consecutive heads overlap.
"""
import os

import numpy as np
import ml_dtypes

import concourse.mybir as mybir
import concourse.tile as tile
from concourse import bacc
import concourse.bass_utils as _bu
from concourse.bass_utils import run_bass_kernel_spmd
from concourse.masks import make_identity

if os.environ.get("BASS_LDW_OPT") and not getattr(_bu, "_ldw_patched", False):
    _orig_run_command = _bu.run_command

    def _run_command_ldw(argv, **kw):
        argv = ["--enable-ldw-opt=true" if a == "--enable-ldw-opt=false"
                else a for a in argv]
        return _orig_run_command(argv, **kw)

    _bu.run_command = _run_command_ldw
    _bu._ldw_patched = True

N_CORES = 8
B, S, D = 1, 2048, 4096
H, KVH, HD = 32, 8, 128
HL = H // N_CORES          # 4 local q heads
SCALE = HD ** -0.5
P = 128
QC = 512                   # attention q-chunk width
NQC = S // QC              # 4
NKB = S // P               # 16 k-tiles
NDC = D // P               # 32 D-chunks
PW = 512                   # phase-1 s-panel width
NPAN = S // PW             # 4
SHARD = QC // N_CORES      # 64 rows per core per RS chunk

FDT = mybir.dt.float32
BDT = mybir.dt.bfloat16
HDT = mybir.dt.float16
NEG = -1.0e9

LAST_RESULT = None


def _build(mode):
    nc = bacc.Bacc("TRN2", target_bir_lowering=False, debug=False,
                   num_devices=N_CORES)
    xt_ext = nc.dram_tensor("xt", [D, S], BDT, kind="ExternalInput")
    wqt_ext = nc.dram_tensor("wqt", [D, HL * P], BDT, kind="ExternalInput")
    wkt_ext = nc.dram_tensor("wkt", [D, P], BDT, kind="ExternalInput")
    wvt_ext = nc.dram_tensor("wvt", [D, P], BDT, kind="ExternalInput")
    wot_ext = nc.dram_tensor("wot", [HL * P, D], BDT, kind="ExternalInput")
    c2_ext = nc.dram_tensor("c2", [P, S], BDT, kind="ExternalInput")
    s2_ext = nc.dram_tensor("s2", [P, S], BDT, kind="ExternalInput")
    if mode == "causal":
        tri_ext = nc.dram_tensor("tri", [P, 896], FDT, kind="ExternalInput")
    if mode == "mask":
        maskt_ext = nc.dram_tensor("maskt", [S, S], FDT, kind="ExternalInput")
    out_ext = nc.dram_tensor("out", [NQC * SHARD, D], HDT,
                             kind="ExternalOutput")

    Alu = mybir.AluOpType
    Act = mybir.ActivationFunctionType

    with tile.TileContext(nc) as tc:
        with tc.tile_pool(name="persist", bufs=1) as pers:
            # per-panel tiles keep dependency tracking fine-grained so the
            # first attention chunks unblock as soon as their panel's rope
            # is done rather than waiting on the whole projection phase
            qt_ps = [pers.tile([P, HL * PW], BDT, tag=f"qt{p}",
                               name=f"qt{p}") for p in range(NPAN)]
            ones_sb = pers.tile([P, P], BDT, tag="ones")
            nc.vector.memset(ones_sb[:], 1.0)
            ident = pers.tile([P, P], BDT, tag="ident")
            make_identity(nc, ident[:])
            kt_ps = [pers.tile([P, PW], BDT, tag=f"kt{p}",
                               name=f"kt{p}") for p in range(NPAN)]
            v_ps = [pers.tile([P, 4 * P], BDT, tag=f"v{p}",
                              name=f"v{p}") for p in range(NPAN)]
            wot_sbs = [pers.tile([P, D], BDT, tag=f"wot{hc}",
                                 name=f"wot{hc}") for hc in range(HL)]
            if mode == "causal":
                tri_sb = pers.tile([P, 896], FDT, tag="tri")
                nc.sync.dma_start(tri_sb[:], tri_ext[:])

            # ---------------- phase 1: QKV projections + RoPE -------------
            with (
                tc.tile_pool(name="ph1", bufs=1) as ph1,
                tc.tile_pool(name="xstage", bufs=2) as xst,
                tc.tile_pool(name="rsc", bufs=2) as rsc,
                tc.tile_pool(name="ppsum", bufs=1, space="PSUM") as ppsum,
                tc.tile_pool(name="tpsum", bufs=2, space="PSUM") as tpsum,
            ):
                HW = HL * P
                wqt_sbs = [ph1.tile([P, 4 * HW], BDT, tag=f"wqt{g}",
                                    name=f"wqt{g}") for g in range(8)]
                wkt_sbs = [ph1.tile([P, 8 * P], BDT, tag=f"wkt{g}",
                                    name=f"wkt{g}") for g in range(4)]
                wvt_sbs = [ph1.tile([P, 8 * P], BDT, tag=f"wvt{g}",
                                    name=f"wvt{g}") for g in range(4)]
                c2_sb = ph1.tile([P, S], BDT, tag="c2")
                s2_sb = ph1.tile([P, S], BDT, tag="s2")

                def dma_weights(dc):
                    if dc == 0:
                        # dc0 slices land first so the opening matmuls
                        # aren't gated on the full 2MB group transfers
                        HW4 = HL * P
                        nc.sync.dma_start(wqt_sbs[0][:, 0:HW4],
                                          wqt_ext[0:P, :])
                        nc.sync.dma_start(wkt_sbs[0][:, 0:P],
                                          wkt_ext[0:P, :])
                        nc.sync.dma_start(wvt_sbs[0][:, 0:P],
                                          wvt_ext[0:P, :])
                        nc.sync.dma_start(
                            wqt_sbs[0][:, HW4:4 * HW4].rearrange(
                                "p (dc h) -> p dc h", dc=3),
                            wqt_ext[P:4 * P, :].rearrange(
                                "(dc p) h -> p dc h", p=P),
                        )
                        nc.sync.dma_start(
                            wkt_sbs[0][:, P:8 * P].rearrange(
                                "p (dc h) -> p dc h", dc=7),
                            wkt_ext[P:8 * P, :].rearrange(
                                "(dc p) h -> p dc h", p=P),
                        )
                        nc.sync.dma_start(
                            wvt_sbs[0][:, P:8 * P].rearrange(
                                "p (dc h) -> p dc h", dc=7),
                            wvt_ext[P:8 * P, :].rearrange(
                                "(dc p) h -> p dc h", p=P),
                        )
                        return
                    if dc % 4 == 0:
                        g = dc // 4
                        nc.sync.dma_start(
                            wqt_sbs[g][:].rearrange(
                                "p (dc h) -> p dc h", dc=4),
                            wqt_ext[g * 4 * P:(g + 1) * 4 * P, :].rearrange(
                                "(dc p) h -> p dc h", p=P),
                        )
                    if dc % 8 == 0:
                        g = dc // 8
                        nc.sync.dma_start(
                            wkt_sbs[g][:].rearrange(
                                "p (dc h) -> p dc h", dc=8),
                            wkt_ext[g * 8 * P:(g + 1) * 8 * P, :].rearrange(
                                "(dc p) h -> p dc h", p=P),
                        )
                        nc.sync.dma_start(
                            wvt_sbs[g][:].rearrange(
                                "p (dc h) -> p dc h", dc=8),
                            wvt_ext[g * 8 * P:(g + 1) * 8 * P, :].rearrange(
                                "(dc p) h -> p dc h", p=P),
                        )

                swap_mask = list(range(16, 32)) + list(range(16))

                def rope_free_accs(pan, qacc, kacc, vacc):
                    """Phase a: everything that READS the PSUM accumulators,
                    so they free up for the next panel ASAP."""
                    cols = slice(pan * PW, (pan + 1) * PW)
                    state = []
                    for i, acc in enumerate(qacc + [kacc]):
                        t_sb = rsc.tile([P, PW], BDT, tag=f"ropea{i}",
                                        name="t_sb")
                        nc.scalar.copy(t_sb[:], acc[:])
                        tsw = rsc.tile([P, PW], BDT, tag=f"ropet{i}",
                                       name="tsw")
                        nc.sync.dma_start(tsw[0:64, :], t_sb[64:128, :])
                        nc.sync.dma_start(tsw[64:128, :], t_sb[0:64, :])
                        m = rsc.tile([P, PW], FDT, tag=f"ropem{i}", name="m")
                        nc.vector.tensor_tensor(m[:], acc[:],
                                                c2_sb[:, cols], op=Alu.mult)
                        state.append((tsw, m))
                    vtmp = rsc.tile([P, PW], BDT, tag="vtmp")
                    nc.scalar.copy(vtmp[:], vacc[:])
                    return (pan, state, vtmp)

                def rope_panel(pan, state, vtmp):
                    cols = slice(pan * PW, (pan + 1) * PW)
                    outs = [qt_ps[pan][:, h * PW:(h + 1) * PW]
                            for h in range(HL)]
                    outs.append(kt_ps[pan][:])
                    for (tsw, m), out in zip(state, outs):
                        n = rsc.tile([P, PW], FDT, tag="ropen")
                        nc.vector.tensor_tensor(n[:], tsw[:],
                                                s2_sb[:, cols], op=Alu.mult)
                        nc.vector.tensor_tensor(out, m[:], n[:], op=Alu.add)
                    vtp = tpsum.tile([P, PW], BDT, tag="xtp", name="vtp")
                    for st4 in range(4):
                        nc.tensor.transpose(
                            vtp[:, st4 * P:(st4 + 1) * P],
                            vtmp[:, st4 * P:(st4 + 1) * P], ident[:])
                    nc.scalar.copy(v_ps[pan][:], vtp[:])

                prev = None
                for pan in range(NPAN):
                    if prev is not None:
                        prev = rope_free_accs(*prev)
                    xts_tiles = []
                    for dc in range(NDC):
                        xts = xst.tile([P, PW], BDT, tag="xts", bufs=33)
                        nc.sync.dma_start(
                            xts[:],
                            xt_ext[dc * P:(dc + 1) * P,
                                   pan * PW:(pan + 1) * PW])
                        if pan == 0:
                            dma_weights(dc)
                        xts_tiles.append(xts)
                    if pan == 0:
                        nc.sync.dma_start(c2_sb[:], c2_ext[:])
                        nc.sync.dma_start(s2_sb[:], s2_ext[:])
                    if pan == 1:
                        # prefetch wo shards while the DMA queues have slack
                        for hc in range(HL):
                            nc.sync.dma_start(
                                wot_sbs[hc][:],
                                wot_ext[hc * P:(hc + 1) * P, :])
                    if prev is not None:
                        rope_panel(*prev)
                    qacc = [ppsum.tile([P, PW], FDT, tag=f"qacc{h}",
                                       name=f"qacc{h}")
                            for h in range(HL)]
                    kacc = ppsum.tile([P, PW], FDT, tag="kacc")
                    vacc = ppsum.tile([P, PW], FDT, tag="vacc")
                    for dc in range(NDC):
                        xts = xts_tiles[dc]
                        first, last = dc == 0, dc == NDC - 1
                        wq_t = wqt_sbs[dc // 4]
                        dq = dc % 4
                        for h in range(HL):
                            nc.tensor.matmul(
                                qacc[h][:],
                                wq_t[:, dq * HW + h * P:
                                     dq * HW + (h + 1) * P],
                                xts[:], start=first, stop=last)
                        nc.tensor.matmul(
                            kacc[:],
                            wkt_sbs[dc // 8][:, (dc % 8) * P:
                                             (dc % 8 + 1) * P],
                            xts[:], start=first, stop=last)
                        nc.tensor.matmul(
                            vacc[:],
                            wvt_sbs[dc // 8][:, (dc % 8) * P:
                                             (dc % 8 + 1) * P],
                            xts[:], start=first, stop=last)
                    prev = (pan, qacc, kacc, vacc)
                rope_panel(*rope_free_accs(*prev))

            # ---------------- phase 2: attention + wo + ReduceScatter -----
            with (
                tc.tile_pool(name="att", bufs=2) as att,
                tc.tile_pool(name="ptpool", bufs=4) as ptp,
                tc.tile_pool(name="scr", bufs=2) as scp,
                tc.tile_pool(name="mtpool", bufs=17) as mtp,
                tc.tile_pool(name="cdram", bufs=4, space="DRAM") as cdram,
                tc.tile_pool(name="stps", bufs=2, space="PSUM") as stps,
                tc.tile_pool(name="avps", bufs=2, space="PSUM") as avps,
                tc.tile_pool(name="wops", bufs=2, space="PSUM") as wops,
            ):
                def attn_chunk(qabs0, qw, mts):
                    """Attention for q columns [qabs0, qabs0+qw); returns
                    per-head normalized (P@V)^T tiles (valid cols [0, qw))."""
                    nkb_ = ((qabs0 + qw) // P if mode == "causal" else NKB)
                    qpan, qpoff = divmod(qabs0, PW)
                    attn_t = []
                    for h in range(HL):
                        ssum = avps.tile([P, QC], FDT, tag="ssum", bufs=2)
                        avt = avps.tile([P, QC], FDT, tag="avt", bufs=2)
                        q0 = h * PW + qpoff
                        for kb in range(nkb_):
                            # causal: piece columns below c0 are fully masked
                            c0 = (max(0, kb * P - qabs0)
                                  if mode == "causal" else 0)
                            st_ps = stps.tile([P, QC], FDT, tag="st")
                            nc.tensor.matmul(
                                st_ps[:, c0:qw],
                                kt_ps[kb // 4][:, (kb % 4) * P:
                                               (kb % 4 + 1) * P],
                                qt_ps[qpan][:, q0 + c0:q0 + qw],
                                start=True, stop=True)
                            if mode == "causal" and kb * P >= qabs0:
                                nc.vector.tensor_tensor(
                                    st_ps[:, c0:c0 + P], st_ps[:, c0:c0 + P],
                                    tri_sb[:, 384:384 + P],
                                    op=Alu.add)
                            elif mode == "mask":
                                nc.vector.tensor_tensor(
                                    st_ps[:, :qw], st_ps[:, :qw],
                                    mts[kb][:, :qw], op=Alu.add)
                            pt = ptp.tile([P, QC], BDT, tag="pt")
                            nc.scalar.activation(pt[:, c0:qw],
                                                 st_ps[:, c0:qw], Act.Exp,
                                                 scale=float(SCALE))
                            first, last = kb == 0, kb == nkb_ - 1
                            nc.tensor.matmul(
                                ssum[:, c0:qw], ones_sb[:], pt[:, c0:qw],
                                start=first, stop=last)
                            nc.tensor.matmul(
                                avt[:, c0:qw],
                                v_ps[kb // 4][:, (kb % 4) * P:
                                              (kb % 4 + 1) * P],
                                pt[:, c0:qw],
                                start=first, stop=last)
                        rsb = scp.tile([P, QC], FDT, tag="rsb")
                        nc.vector.reciprocal_approx_fast(out=rsb[:, :qw],
                                                         in_=ssum[:, :qw])
                        at = att.tile([P, QC], BDT, tag=f"attnT{h}",
                                      name=f"attnT{h}")
                        nc.vector.tensor_tensor(at[:, :qw], avt[:, :qw],
                                                rsb[:, :qw], op=Alu.mult)
                        attn_t.append(at)
                    return attn_t

                def wo_rs(attn_t, col0, cnt, out_off, last=False):
                    """wo partials for cnt 128-row blocks of attn_t starting
                    at column col0, then an eagerly-issued ReduceScatter
                    straight to the output."""
                    hrows = cnt * P
                    rs_in = cdram.tile([hrows, D], HDT, tag="rsin",
                                       bufs=4, name="rs_in",
                                       padded_shape=[QC, D])
                    for st2 in range(cnt):
                        rs_full = scp.tile([P, D], HDT, tag="rsfull",
                                           bufs=4, name="rs_full")
                        for do in range(8):
                            ops = wops.tile([P, QC], FDT, tag="wops",
                                            bufs=2)
                            for hc in range(HL):
                                nc.tensor.matmul(
                                    ops[:],
                                    attn_t[hc][:, col0 + st2 * P:
                                               col0 + (st2 + 1) * P],
                                    wot_sbs[hc][:, do * QC:(do + 1) * QC],
                                    start=(hc == 0), stop=(hc == HL - 1))
                            nc.vector.tensor_copy(
                                out=rs_full[:, do * QC:(do + 1) * QC],
                                in_=ops[:])
                        # one 8KB-per-partition DMA per 128 rows: big
                        # contiguous descriptors survive RS ring contention
                        nc.sync.dma_start(
                            rs_in[st2 * P:(st2 + 1) * P, :], rs_full[:])
                    nr = hrows // N_CORES
                    rs_out = cdram.tile([nr, D], HDT, tag="rsout", bufs=4,
                                        name="rs_out",
                                        padded_shape=[QC // N_CORES, D])
                    nc.gpsimd.collective_compute(
                        "ReduceScatter", Alu.add,
                        ins=[rs_in[:]], outs=[rs_out[:]],
                        replica_groups=[list(range(N_CORES))])
                    # the final piece's output DMA rides the HWDGE sync
                    # queue (lower first-byte latency on the critical tail)
                    eng = nc.sync if last else nc.gpsimd
                    eng.dma_start(
                        out_ext[out_off:out_off + nr, :], rs_out[:])
                    return out_off + nr

                out_off = 0
                for qc in range(NQC):
                    # the final q-chunk runs in 256-row pieces so the last
                    # ReduceScatter on the critical tail is small (256 keeps
                    # the matmul free dim above the LDWEIGHTS-bound regime)
                    pieces = ([(0, QC)] if qc != NQC - 1
                              else [(0, 2 * P), (2 * P, 2 * P)])
                    for qoff, qw in pieces:
                        qabs0 = qc * QC + qoff
                        mts = []
                        if mode == "mask":
                            for kb in range(NKB):
                                mt = mtp.tile([P, QC], FDT, tag="mt",
                                              name="mt")
                                nc.sync.dma_start(
                                    mt[:, :qw],
                                    maskt_ext[kb * P:(kb + 1) * P,
                                              qabs0:qabs0 + qw])
                                mts.append(mt)
                        attn_t = attn_chunk(qabs0, qw, mts)
                        out_off = wo_rs(attn_t, 0, qw // P, out_off,
                                        last=(qc == NQC - 1
                                              and qoff == 2 * P))
    nc.compile()
    return nc


def _prep_inputs(x, freqs_cos, freqs_sin, mask, wq, wk, wv, wo, mode):
    bf16 = ml_dtypes.bfloat16
    perm = np.concatenate([np.arange(0, HD, 2), np.arange(1, HD, 2)])
    xt = np.ascontiguousarray(x.reshape(S, D).T.astype(bf16))
    cosT = np.ascontiguousarray(freqs_cos.T, dtype=np.float32)  # (64, S)
    sinT = np.ascontiguousarray(freqs_sin.T, dtype=np.float32)
    c2 = np.ascontiguousarray(np.vstack([cosT, cosT]).astype(bf16))
    s2 = np.ascontiguousarray(np.vstack([-sinT, sinT]).astype(bf16))
    t = np.arange(896) - 384
    tri = np.where(t[None, :] >= np.arange(P)[:, None], 0.0,
                   NEG / SCALE).astype(np.float32)
    wq4 = wq.reshape(H, HD, D)[:, perm, :]
    wk4 = wk.reshape(KVH, HD, D)[:, perm, :]
    wv4 = wv.reshape(KVH, HD, D)
    in_maps = []
    for c in range(N_CORES):
        wqs = wq4[c * HL:(c + 1) * HL].reshape(HL * HD, D)
        m = {
            "xt": xt,
            "wqt": np.ascontiguousarray(wqs.T).astype(bf16),
            "wkt": np.ascontiguousarray(wk4[c].T).astype(bf16),
            "wvt": np.ascontiguousarray(wv4[c].T).astype(bf16),
            "wot": np.ascontiguousarray(
                wo[:, c * HL * HD:(c + 1) * HL * HD].T).astype(bf16),
            "c2": c2, "s2": s2,
        }
        if mode == "causal":
            m["tri"] = tri
        if mode == "mask":
            m["maskt"] = np.ascontiguousarray(
                mask.T / SCALE, dtype=np.float32)
        in_maps.append(m)
    return in_maps


def _mask_mode(mask):
    if np.all(mask == 0):
        return "zeros"
    iu = np.triu_indices(S, 1)
    if (np.all(np.tril(mask) == 0) and np.all(mask[iu] <= -1e8)
            and np.all(mask[iu] >= -2e9)):
        return "causal"
    return "mask"


_GRAPH_CACHE = {}


def kernel(x, freqs_cos, freqs_sin, mask, wq, wk, wv, wo):
    global LAST_RESULT
    mode = _mask_mode(np.asarray(mask))
    if mode not in _GRAPH_CACHE:
        _GRAPH_CACHE[mode] = _build(mode)
    nc = _GRAPH_CACHE[mode]
    in_maps = _prep_inputs(
        np.asarray(x), np.asarray(freqs_cos), np.asarray(freqs_sin),
        np.asarray(mask), np.asarray(wq), np.asarray(wk), np.asarray(wv),
        np.asarray(wo), mode)
    res = run_bass_kernel_spmd(
        nc, in_maps, core_ids=list(range(N_CORES)),
        trace=bool(os.environ.get("BASS_TRACE")))
    LAST_RESULT = res
    out = np.empty((S, D), dtype=np.float32)
    chunks = [(0, 512), (512, 512), (1024, 512), (1536, 256), (1792, 256)]
    for c in range(N_CORES):
        shard = np.asarray(res.results[c]["out"], dtype=np.float32)
        off = 0
        for src_row0, nrows in chunks:
            nr = nrows // N_CORES
            out[src_row0 + c * nr: src_row0 + (c + 1) * nr] = \
                shard[off:off + nr]
            off += nr
    return out.reshape(B, S, D)



# revision 24
# speedup vs baseline: 1.0341x; 1.0067x over previous
"""Tensor-parallel GQA attention block for 8 TRN2 NeuronCores.

Sharding: TP over heads. Core c owns query heads 4c..4c+3 and KV head c
(column-shard of wq/wk/wv), plus the matching column-shard of wo. x is
replicated. Each core computes its partial x@wo_shard.T contribution and a
ReduceScatter sums them, leaving each core with a row-shard of the output;
the host concatenates the shards.

All device compute in bf16 with f32 PSUM accumulation. Host-side input prep:
weight shards are transposed to contraction-major layout, query-head dims are
permuted to (evens, odds) so RoPE halves sit in partition halves.

Scores are computed transposed (S^T = K @ Q^T, [k, q] layout) so the exp'd
probabilities feed the P@V matmul directly as the moving operand; the
1/sqrt(HD) scale is applied inside the exp activation. Softmax denominators
come from an all-ones matmul (sum + broadcast across partitions in one op)
and are applied as a reciprocal multiply. exp() runs without max-subtraction:
scores here are bounded (|score| < ~15), safe in f32.

The wo partials ship to the ReduceScatter in five chunks (three 512-row for
q-chunks 0-2, then two 256-row pieces for the final q-chunk so the collective
on the critical tail is smaller; 256 keeps matmul free dims above the
LDWEIGHTS-bound regime), each issued eagerly. Partial rows are assembled to
full 8KB-per-partition lines in SBUF before the DRAM staging write so the
descriptors survive contention with the collective's ring traffic, and the
staging buffers are deep enough (bufs=4) to ride out that contention without
stalling the PE. wo weight shards prefetch during phase 1, the softmax
accumulators are double-buffered so consecutive heads overlap, qt/kt/v live
in per-panel tiles for fine-grained dependencies, and the first projection
matmuls get dedicated dc0 weight-slice DMAs so they start early. The final
piece's output DMA rides the HWDGE sync queue; earlier ones ride GpSimd.
"""
import os

import numpy as np
import ml_dtypes

import concourse.mybir as mybir
import concourse.tile as tile
from concourse import bacc
import concourse.bass_utils as _bu
from concourse.bass_utils import run_bass_kernel_spmd
from concourse.masks import make_identity

if os.environ.get("BASS_LDW_OPT") and not getattr(_bu, "_ldw_patched", False):
    _orig_run_command = _bu.run_command

    def _run_command_ldw(argv, **kw):
        argv = ["--enable-ldw-opt=true" if a == "--enable-ldw-opt=false"
                else a for a in argv]
        return _orig_run_command(argv, **kw)

    _bu.run_command = _run_command_ldw
    _bu._ldw_patched = True

N_CORES = 8
B, S, D = 1, 2048, 4096
H, KVH, HD = 32, 8, 128
HL = H // N_CORES          # 4 local q heads
SCALE = HD ** -0.5
P = 128
QC = 512                   # attention q-chunk width
NQC = S // QC              # 4
NKB = S // P               # 16 k-tiles
NDC = D // P               # 32 D-chunks
PW = 512                   # phase-1 s-panel width
NPAN = S // PW             # 4
SHARD = QC // N_CORES      # 64 rows per core per RS chunk

FDT = mybir.dt.float32
BDT = mybir.dt.bfloat16
HDT = mybir.dt.float16
NEG = -1.0e9

LAST_RESULT = None


def _build(mode):
    nc = bacc.Bacc("TRN2", target_bir_lowering=False, debug=False,
                   num_devices=N_CORES)
    xt_ext = nc.dram_tensor("xt", [D, S], BDT, kind="ExternalInput")
    wqt_ext = nc.dram_tensor("wqt", [D, HL * P], BDT, kind="ExternalInput")
    wkt_ext = nc.dram_tensor("wkt", [D, P], BDT, kind="ExternalInput")
    wvt_ext = nc.dram_tensor("wvt", [D, P], BDT, kind="ExternalInput")
    wot_ext = nc.dram_tensor("wot", [HL * P, D], BDT, kind="ExternalInput")
    c2_ext = nc.dram_tensor("c2", [P, S], BDT, kind="ExternalInput")
    s2_ext = nc.dram_tensor("s2", [P, S], BDT, kind="ExternalInput")
    if mode == "causal":
        tri_ext = nc.dram_tensor("tri", [P, 896], FDT, kind="ExternalInput")
    if mode == "mask":
        maskt_ext = nc.dram_tensor("maskt", [S, S], FDT, kind="ExternalInput")
    out_ext = nc.dram_tensor("out", [NQC * SHARD, D], HDT,
                             kind="ExternalOutput")

    Alu = mybir.AluOpType
    Act = mybir.ActivationFunctionType

    with tile.TileContext(nc) as tc:
        with tc.tile_pool(name="persist", bufs=1) as pers:
            # per-panel tiles keep dependency tracking fine-grained so the
            # first attention chunks unblock as soon as their panel's rope
            # is done rather than waiting on the whole projection phase
            qt_ps = [pers.tile([P, HL * PW], BDT, tag=f"qt{p}",
                               name=f"qt{p}") for p in range(NPAN)]
            ones_sb = pers.tile([P, P], BDT, tag="ones")
            nc.vector.memset(ones_sb[:], 1.0)
            ident = pers.tile([P, P], BDT, tag="ident")
            make_identity(nc, ident[:])
            kt_ps = [pers.tile([P, PW], BDT, tag=f"kt{p}",
                               name=f"kt{p}") for p in range(NPAN)]
            v_ps = [pers.tile([P, 4 * P], BDT, tag=f"v{p}",
                              name=f"v{p}") for p in range(NPAN)]
            wot_sbs = [pers.tile([P, D], BDT, tag=f"wot{hc}",
                                 name=f"wot{hc}") for hc in range(HL)]
            if mode == "causal":
                tri_sb = pers.tile([P, 896], FDT, tag="tri")
                nc.sync.dma_start(tri_sb[:], tri_ext[:])

            # ---------------- phase 1: QKV projections + RoPE -------------
            with (
                tc.tile_pool(name="ph1", bufs=1) as ph1,
                tc.tile_pool(name="xstage", bufs=2) as xst,
                tc.tile_pool(name="rsc", bufs=2) as rsc,
                tc.tile_pool(name="ppsum", bufs=1, space="PSUM") as ppsum,
                tc.tile_pool(name="tpsum", bufs=2, space="PSUM") as tpsum,
            ):
                HW = HL * P
                wqt_sbs = [ph1.tile([P, 4 * HW], BDT, tag=f"wqt{g}",
                                    name=f"wqt{g}") for g in range(8)]
                wkt_sbs = [ph1.tile([P, 8 * P], BDT, tag=f"wkt{g}",
                                    name=f"wkt{g}") for g in range(4)]
                wvt_sbs = [ph1.tile([P, 8 * P], BDT, tag=f"wvt{g}",
                                    name=f"wvt{g}") for g in range(4)]
                c2_sb = ph1.tile([P, S], BDT, tag="c2")
                s2_sb = ph1.tile([P, S], BDT, tag="s2")

                def dma_weights(dc):
                    if dc == 0:
                        # dc0 slices land first so the opening matmuls
                        # aren't gated on the full 2MB group transfers
                        HW4 = HL * P
                        nc.sync.dma_start(wqt_sbs[0][:, 0:HW4],
                                          wqt_ext[0:P, :])
                        nc.sync.dma_start(wkt_sbs[0][:, 0:P],
                                          wkt_ext[0:P, :])
                        nc.sync.dma_start(wvt_sbs[0][:, 0:P],
                                          wvt_ext[0:P, :])
                        nc.sync.dma_start(
                            wqt_sbs[0][:, HW4:4 * HW4].rearrange(
                                "p (dc h) -> p dc h", dc=3),
                            wqt_ext[P:4 * P, :].rearrange(
                                "(dc p) h -> p dc h", p=P),
                        )
                        nc.sync.dma_start(
                            wkt_sbs[0][:, P:8 * P].rearrange(
                                "p (dc h) -> p dc h", dc=7),
                            wkt_ext[P:8 * P, :].rearrange(
                                "(dc p) h -> p dc h", p=P),
                        )
                        nc.sync.dma_start(
                            wvt_sbs[0][:, P:8 * P].rearrange(
                                "p (dc h) -> p dc h", dc=7),
                            wvt_ext[P:8 * P, :].rearrange(
                                "(dc p) h -> p dc h", p=P),
                        )
                        return
                    if dc % 4 == 0:
                        g = dc // 4
                        nc.sync.dma_start(
                            wqt_sbs[g][:].rearrange(
                                "p (dc h) -> p dc h", dc=4),
                            wqt_ext[g * 4 * P:(g + 1) * 4 * P, :].rearrange(
                                "(dc p) h -> p dc h", p=P),
                        )
                    if dc % 8 == 0:
                        g = dc // 8
                        nc.sync.dma_start(
                            wkt_sbs[g][:].rearrange(
                                "p (dc h) -> p dc h", dc=8),
                            wkt_ext[g * 8 * P:(g + 1) * 8 * P, :].rearrange(
                                "(dc p) h -> p dc h", p=P),
                        )
                        nc.sync.dma_start(
                            wvt_sbs[g][:].rearrange(
                                "p (dc h) -> p dc h", dc=8),
                            wvt_ext[g * 8 * P:(g + 1) * 8 * P, :].rearrange(
                                "(dc p) h -> p dc h", p=P),
                        )

                swap_mask = list(range(16, 32)) + list(range(16))

                def rope_free_accs(pan, qacc, kacc, vacc):
                    """Phase a: everything that READS the PSUM accumulators,
                    so they free up for the next panel ASAP."""
                    cols = slice(pan * PW, (pan + 1) * PW)
                    state = []
                    for i, acc in enumerate(qacc + [kacc]):
                        t_sb = rsc.tile([P, PW], BDT, tag=f"ropea{i}",
                                        name="t_sb")
                        nc.scalar.copy(t_sb[:], acc[:])
                        tsw = rsc.tile([P, PW], BDT, tag=f"ropet{i}",
                                       name="tsw")
                        nc.sync.dma_start(tsw[0:64, :], t_sb[64:128, :])
                        nc.sync.dma_start(tsw[64:128, :], t_sb[0:64, :])
                        m = rsc.tile([P, PW], FDT, tag=f"ropem{i}", name="m")
                        nc.vector.tensor_tensor(m[:], acc[:],
                                                c2_sb[:, cols], op=Alu.mult)
                        state.append((tsw, m))
                    vtmp = rsc.tile([P, PW], BDT, tag="vtmp")
                    nc.scalar.copy(vtmp[:], vacc[:])
                    return (pan, state, vtmp)

                def rope_panel(pan, state, vtmp):
                    cols = slice(pan * PW, (pan + 1) * PW)
                    outs = [qt_ps[pan][:, h * PW:(h + 1) * PW]
                            for h in range(HL)]
                    outs.append(kt_ps[pan][:])
                    for (tsw, m), out in zip(state, outs):
                        n = rsc.tile([P, PW], FDT, tag="ropen")
                        nc.vector.tensor_tensor(n[:], tsw[:],
                                                s2_sb[:, cols], op=Alu.mult)
                        nc.vector.tensor_tensor(out, m[:], n[:], op=Alu.add)
                    vtp = tpsum.tile([P, PW], BDT, tag="xtp", name="vtp")
                    for st4 in range(4):
                        nc.tensor.transpose(
                            vtp[:, st4 * P:(st4 + 1) * P],
                            vtmp[:, st4 * P:(st4 + 1) * P], ident[:])
                    nc.scalar.copy(v_ps[pan][:], vtp[:])

                prev = None
                for pan in range(NPAN):
                    if prev is not None:
                        prev = rope_free_accs(*prev)
                    xts_tiles = []
                    for dc in range(NDC):
                        xts = xst.tile([P, PW], BDT, tag="xts", bufs=33)
                        nc.sync.dma_start(
                            xts[:],
                            xt_ext[dc * P:(dc + 1) * P,
                                   pan * PW:(pan + 1) * PW])
                        if pan == 0:
                            dma_weights(dc)
                        xts_tiles.append(xts)
                    if pan == 0:
                        nc.sync.dma_start(c2_sb[:], c2_ext[:])
                        nc.sync.dma_start(s2_sb[:], s2_ext[:])
                    if pan == 1:
                        # prefetch wo shards while the DMA queues have slack
                        for hc in range(HL):
                            nc.sync.dma_start(
                                wot_sbs[hc][:],
                                wot_ext[hc * P:(hc + 1) * P, :])
                    if prev is not None:
                        rope_panel(*prev)
                    qacc = [ppsum.tile([P, PW], FDT, tag=f"qacc{h}",
                                       name=f"qacc{h}")
                            for h in range(HL)]
                    kacc = ppsum.tile([P, PW], FDT, tag="kacc")
                    vacc = ppsum.tile([P, PW], FDT, tag="vacc")
                    for dc in range(NDC):
                        xts = xts_tiles[dc]
                        first, last = dc == 0, dc == NDC - 1
                        wq_t = wqt_sbs[dc // 4]
                        dq = dc % 4
                        for h in range(HL):
                            nc.tensor.matmul(
                                qacc[h][:],
                                wq_t[:, dq * HW + h * P:
                                     dq * HW + (h + 1) * P],
                                xts[:], start=first, stop=last)
                        nc.tensor.matmul(
                            kacc[:],
                            wkt_sbs[dc // 8][:, (dc % 8) * P:
                                             (dc % 8 + 1) * P],
                            xts[:], start=first, stop=last)
                        nc.tensor.matmul(
                            vacc[:],
                            wvt_sbs[dc // 8][:, (dc % 8) * P:
                                             (dc % 8 + 1) * P],
                            xts[:], start=first, stop=last)
                    prev = (pan, qacc, kacc, vacc)
                rope_panel(*rope_free_accs(*prev))

            # ---------------- phase 2: attention + wo + ReduceScatter -----
            with (
                tc.tile_pool(name="att", bufs=2) as att,
                tc.tile_pool(name="ptpool", bufs=4) as ptp,
                tc.tile_pool(name="scr", bufs=2) as scp,
                tc.tile_pool(name="mtpool", bufs=17) as mtp,
                tc.tile_pool(name="cdram", bufs=4, space="DRAM") as cdram,
                tc.tile_pool(name="stps", bufs=2, space="PSUM") as stps,
                tc.tile_pool(name="avps", bufs=2, space="PSUM") as avps,
                tc.tile_pool(name="wops", bufs=2, space="PSUM") as wops,
            ):
                def attn_chunk(qabs0, qw, mts):
                    """Attention for q columns [qabs0, qabs0+qw); returns
                    per-head normalized (P@V)^T tiles (valid cols [0, qw))."""
                    nkb_ = ((qabs0 + qw) // P if mode == "causal" else NKB)
                    qpan, qpoff = divmod(qabs0, PW)
                    attn_t = []
                    for h in range(HL):
                        ssum = avps.tile([P, QC], FDT, tag="ssum", bufs=2)
                        avt = avps.tile([P, QC], FDT, tag="avt", bufs=2)
                        q0 = h * PW + qpoff
                        for kb in range(nkb_):
                            # causal: piece columns below c0 are fully masked
                            c0 = (max(0, kb * P - qabs0)
                                  if mode == "causal" else 0)
                            st_ps = stps.tile([P, QC], FDT, tag="st")
                            nc.tensor.matmul(
                                st_ps[:, c0:qw],
                                kt_ps[kb // 4][:, (kb % 4) * P:
                                               (kb % 4 + 1) * P],
                                qt_ps[qpan][:, q0 + c0:q0 + qw],
                                start=True, stop=True)
                            if mode == "causal" and kb * P >= qabs0:
                                nc.vector.tensor_tensor(
                                    st_ps[:, c0:c0 + P], st_ps[:, c0:c0 + P],
                                    tri_sb[:, 384:384 + P],
                                    op=Alu.add)
                            elif mode == "mask":
                                nc.vector.tensor_tensor(
                                    st_ps[:, :qw], st_ps[:, :qw],
                                    mts[kb][:, :qw], op=Alu.add)
                            pt = ptp.tile([P, QC], BDT, tag="pt")
                            nc.scalar.activation(pt[:, c0:qw],
                                                 st_ps[:, c0:qw], Act.Exp,
                                                 scale=float(SCALE))
                            first, last = kb == 0, kb == nkb_ - 1
                            nc.tensor.matmul(
                                ssum[:, c0:qw], ones_sb[:], pt[:, c0:qw],
                                start=first, stop=last)
                            nc.tensor.matmul(
                                avt[:, c0:qw],
                                v_ps[kb // 4][:, (kb % 4) * P:
                                              (kb % 4 + 1) * P],
                                pt[:, c0:qw],
                                start=first, stop=last)
                        rsb = scp.tile([P, QC], FDT, tag="rsb")
                        nc.vector.reciprocal_approx_fast(out=rsb[:, :qw],
                                                         in_=ssum[:, :qw])
                        at = att.tile([P, QC], BDT, tag=f"attnT{h}",
                                      name=f"attnT{h}")
                        nc.vector.tensor_tensor(at[:, :qw], avt[:, :qw],
                                                rsb[:, :qw], op=Alu.mult)
                        attn_t.append(at)
                    return attn_t

                def wo_rs(attn_t, col0, cnt, out_off, last=False):
                    """wo partials for cnt 128-row blocks of attn_t starting
                    at column col0, then an eagerly-issued ReduceScatter
                    straight to the output."""
                    hrows = cnt * P
                    rs_in = cdram.tile([hrows, D], HDT, tag="rsin",
                                       bufs=4, name="rs_in",
                                       padded_shape=[QC, D])
                    for st2 in range(cnt):
                        rs_full = scp.tile([P, D], HDT, tag="rsfull",
                                           bufs=4, name="rs_full")
                        for do in range(8):
                            ops = wops.tile([P, QC], FDT, tag="wops",
                                            bufs=2)
                            for hc in range(HL):
                                nc.tensor.matmul(
                                    ops[:],
                                    attn_t[hc][:, col0 + st2 * P:
                                               col0 + (st2 + 1) * P],
                                    wot_sbs[hc][:, do * QC:(do + 1) * QC],
                                    start=(hc == 0), stop=(hc == HL - 1))
                            nc.vector.tensor_copy(
                                out=rs_full[:, do * QC:(do + 1) * QC],
                                in_=ops[:])
                        # one 8KB-per-partition DMA per 128 rows: big
                        # contiguous descriptors survive RS ring contention
                        nc.sync.dma_start(
                            rs_in[st2 * P:(st2 + 1) * P, :], rs_full[:])
                    nr = hrows // N_CORES
                    rs_out = cdram.tile([nr, D], HDT, tag="rsout", bufs=4,
                                        name="rs_out",
                                        padded_shape=[QC // N_CORES, D])
                    nc.gpsimd.collective_compute(
                        "ReduceScatter", Alu.add,
                        ins=[rs_in[:]], outs=[rs_out[:]],
                        replica_groups=[list(range(N_CORES))])
                    # the final piece's output DMA rides the HWDGE sync
                    # queue (lower first-byte latency on the critical tail)
                    eng = nc.sync if last else nc.gpsimd
                    eng.dma_start(
                        out_ext[out_off:out_off + nr, :], rs_out[:])
                    return out_off + nr

                out_off = 0
                for qc in range(NQC):
                    # the final q-chunk runs in 256-row pieces so the last
                    # ReduceScatter on the critical tail is small (256 keeps
                    # the matmul free dim above the LDWEIGHTS-bound regime)
                    pieces = ([(0, QC)] if qc != NQC - 1
                              else [(0, 2 * P), (2 * P, 2 * P)])
                    for qoff, qw in pieces:
                        qabs0 = qc * QC + qoff
                        mts = []
                        if mode == "mask":
                            for kb in range(NKB):
                                mt = mtp.tile([P, QC], FDT, tag="mt",
                                              name="mt")
                                nc.sync.dma_start(
                                    mt[:, :qw],
                                    maskt_ext[kb * P:(kb + 1) * P,
                                              qabs0:qabs0 + qw])
                                mts.append(mt)
                        attn_t = attn_chunk(qabs0, qw, mts)
                        out_off = wo_rs(attn_t, 0, qw // P, out_off,
                                        last=(qc == NQC - 1
                                              and qoff == 2 * P))
    nc.compile()
    return nc


def _prep_inputs(x, freqs_cos, freqs_sin, mask, wq, wk, wv, wo, mode):
    bf16 = ml_dtypes.bfloat16
    perm = np.concatenate([np.arange(0, HD, 2), np.arange(1, HD, 2)])
    xt = np.ascontiguousarray(x.reshape(S, D).T.astype(bf16))
    cosT = np.ascontiguousarray(freqs_cos.T, dtype=np.float32)  # (64, S)
    sinT = np.ascontiguousarray(freqs_sin.T, dtype=np.float32)
    c2 = np.ascontiguousarray(np.vstack([cosT, cosT]).astype(bf16))
    s2 = np.ascontiguousarray(np.vstack([-sinT, sinT]).astype(bf16))
    t = np.arange(896) - 384
    tri = np.where(t[None, :] >= np.arange(P)[:, None], 0.0,
                   NEG / SCALE).astype(np.float32)
    wq4 = wq.reshape(H, HD, D)[:, perm, :]
    wk4 = wk.reshape(KVH, HD, D)[:, perm, :]
    wv4 = wv.reshape(KVH, HD, D)
    in_maps = []
    for c in range(N_CORES):
        wqs = wq4[c * HL:(c + 1) * HL].reshape(HL * HD, D)
        m = {
            "xt": xt,
            "wqt": np.ascontiguousarray(wqs.T).astype(bf16),
            "wkt": np.ascontiguousarray(wk4[c].T).astype(bf16),
            "wvt": np.ascontiguousarray(wv4[c].T).astype(bf16),
            "wot": np.ascontiguousarray(
                wo[:, c * HL * HD:(c + 1) * HL * HD].T).astype(bf16),
            "c2": c2, "s2": s2,
        }
        if mode == "causal":
            m["tri"] = tri
        if mode == "mask":
            m["maskt"] = np.ascontiguousarray(
                mask.T / SCALE, dtype=np.float32)
        in_maps.append(m)
    return in_maps


def _mask_mode(mask):
    if np.all(mask == 0):
        return "zeros"
    iu = np.triu_indices(S, 1)
    if (np.all(np.tril(mask) == 0) and np.all(mask[iu] <= -1e8)
            and np.all(mask[iu] >= -2e9)):
        return "causal"
    return "mask"


_GRAPH_CACHE = {}


def kernel(x, freqs_cos, freqs_sin, mask, wq, wk, wv, wo):
    global LAST_RESULT
    mode = _mask_mode(np.asarray(mask))
    if mode not in _GRAPH_CACHE:
        _GRAPH_CACHE[mode] = _build(mode)
    nc = _GRAPH_CACHE[mode]
    in_maps = _prep_inputs(
        np.asarray(x), np.asarray(freqs_cos), np.asarray(freqs_sin),
        np.asarray(mask), np.asarray(wq), np.asarray(wk), np.asarray(wv),
        np.asarray(wo), mode)
    res = run_bass_kernel_spmd(
        nc, in_maps, core_ids=list(range(N_CORES)),
        trace=bool(os.environ.get("BASS_TRACE")))
    LAST_RESULT = res
    out = np.empty((S, D), dtype=np.float32)
    chunks = [(0, 512), (512, 512), (1024, 512), (1536, 256), (1792, 256)]
    for c in range(N_CORES):
        shard = np.asarray(res.results[c]["out"], dtype=np.float32)
        off = 0
        for src_row0, nrows in chunks:
            nr = nrows // N_CORES
            out[src_row0 + c * nr: src_row0 + (c + 1) * nr] = \
                shard[off:off + nr]
            off += nr
    return out.reshape(B, S, D)

